# revision 13
# baseline (speedup 1.0000x reference)
"""BinaryConv2D (3x3 SAME, stride 1, NHWC/HWIO) on 8 Trainium2 NeuronCores.

Strategy
--------
Data-parallel over batch: 32 images -> 4 per core. Weights are binarized
host-side to +/-1 and replicated.

Per core the conv is computed as 18 accumulating float32r matmuls per
output tile (9 taps x 2 input-channel chunks of 128): the binarized
weight chunk [cin=128, cout=128] is the stationary operand, and a
2D-strided window of the channels-first padded image (8 output rows x 56
valid columns, row stride 58) is the moving operand. PSUM accumulates in
fp32; float32r streams at 1 row/cycle for moving dim >= 256 (4x faster
than fp32, ~1.1e-4 rel err for this 2304-term contraction on HW).

Host-side layout work (sharding prep): pad each image to 58x58 with a
zero ring, transpose to channels-first, flatten (img, y, x). A tap shift
is then a single scalar offset into the flat pixel axis, and every tap
read stays inside the padded image. The device output is channels-first
[cout_chunk, 128, img, y, x]; the host transposes back to NHWC.

Head-latency hiding: PE warmup matmuls on scratch data run while inputs
stream in (HAM un-throttles before real work), weights load on the ACT
HW-DGE ring concurrently with x on the SP ring, and image 0 arrives in
two halves so compute starts after ~1MB.
"""
import sys

sys.path.insert(0, "/opt/trn_rl_repo")

import numpy as np

P = 128
H = W = 56
HP = WP = 58                  # padded spatial dims
PIX = HP * WP                 # 3364 padded pixels per image
N_CORES = 8
IMGS = 4                      # images per core
FREE = IMGS * PIX             # 13456
ROWS = 8                      # output rows per tile
TILES = H // ROWS             # 7 tiles per image per cout chunk
NT = ROWS * W                 # 448 moving elements per matmul
KC = 2                        # cin chunks of 128
MC = 2                        # cout chunks of 128
TAPS = 9
WARMUP_MM = 5

_nc_cache = None


def _build_nc():
    import concourse.bacc as bacc
    import concourse.mybir as mybir
    from concourse import tile

    nc = bacc.Bacc("TRN2", target_bir_lowering=False, debug=False)
    f32r = mybir.dt.float32r
    f32 = mybir.dt.float32

    xt_d = nc.dram_tensor("xt", [KC, P, FREE], f32r, kind="ExternalInput")
    wt_d = nc.dram_tensor("wt", [P, MC, KC, TAPS, P], f32r, kind="ExternalInput")
    o_d = nc.dram_tensor("o", [MC, P, IMGS, H, W], f32, kind="ExternalOutput")

    with tile.TileContext(nc) as tc:
        with (
            tc.tile_pool(name="sb", bufs=1) as sb,
            tc.tile_pool(name="ps", bufs=1, space="PSUM") as ps,
        ):
            # PE warmup on zeroed scratch while input DMAs run: HAM
            # un-throttles to full clock before the real matmuls start.
            scratch = sb.tile([P, 512], f32r)
            nc.gpsimd.memset(scratch[:].bitcast(f32), 0.0)
            # One long accumulation group: back-to-back PE activity (no
            # inter-matmul sem roundtrips) so HAM un-throttles during the
            # input DMA head.
            warm = ps.tile([P, 512], f32, name="pt", bufs=8)
            for k in range(WARMUP_MM):
                nc.tensor.matmul(warm[:], scratch[:, :P], scratch[:],
                                 start=(k == 0), stop=(k == WARMUP_MM - 1),
                                 skip_group_check=True)

            w_sb = sb.tile([P, MC, KC, TAPS, P], f32r)
            x_sb = sb.tile([P, KC, FREE], f32r)
            # x on the SP ring in arrival-order pieces: rows 0-25 of
            # image 0 (tiles t=0..2) first, then the rest, then whole
            # images. Weights stream concurrently on the ACT ring in
            # consumption order; with the (img, mc, t) group order below,
            # mc1 weights have ~25us of slack.
            spans = [(0, 26 * WP), (26 * WP, PIX)] + [
                (i * PIX, (i + 1) * PIX) for i in range(1, IMGS)
            ]
            for lo, hi in spans:
                nc.sync.dma_start(
                    x_sb[:, :, lo:hi],
                    xt_d[:, :, lo:hi].rearrange("kc p q -> p kc q"),
                )
            for mc in range(MC):
                for kc in range(KC):
                    nc.scalar.dma_start(w_sb[:, mc, kc], wt_d[:, mc, kc])

            for i in range(IMGS):
                for mc in range(MC):
                    for t in range(TILES):
                        pt = ps.tile([P, ROWS, W], f32, name="pt", bufs=8)
                        k = 0
                        for kc in range(KC):
                            for tap in range(TAPS):
                                dy, dx = tap // 3, tap % 3
                                q0 = i * PIX + (ROWS * t + dy) * WP
                                rhs = (
                                    x_sb[:, kc, q0:q0 + ROWS * WP]
                                    .rearrange("p (r c) -> p r c", r=ROWS, c=WP)
                                    [:, :, dx:dx + W]
                                )
                                nc.tensor.matmul(
                                    pt[:],
                                    w_sb[:, mc, kc, tap, :],
                                    rhs,
                                    start=(k == 0),
                                    stop=(k == TAPS * KC - 1),
                                )
                                k += 1
                        o_t = sb.tile([P, ROWS, W], f32, name="ot", bufs=4)
                        nc.vector.tensor_copy(o_t[:], pt[:])
                        nc.sync.dma_start(
                            o_d[mc, :, i, ROWS * t:ROWS * (t + 1), :],
                            o_t[:],
                        )
    nc.compile()
    return nc


def _get_nc():
    global _nc_cache
    if _nc_cache is None:
        _nc_cache = _build_nc()
    return _nc_cache


def prep_inputs(x: np.ndarray, kernel: np.ndarray):
    """Host-side sharding/layout prep shared by kernel() and test timing."""
    x = np.ascontiguousarray(x, dtype=np.float32)
    kernel = np.ascontiguousarray(kernel, dtype=np.float32)
    assert x.shape == (32, H, W, 256) and kernel.shape == (3, 3, 256, 256)

    wb = np.where(kernel >= 0, np.float32(1.0), np.float32(-1.0))
    wt = (
        wb.reshape(TAPS, KC, P, MC, P)      # [tap, kc, p, mc, m]
        .transpose(2, 3, 1, 0, 4)           # [p, mc, kc, tap, m]
        .copy()
    )

    xt = np.zeros((N_CORES, KC, P, FREE), np.float32)
    reg = xt.reshape(N_CORES, KC, P, IMGS, HP, WP)
    xr = x.reshape(N_CORES, IMGS, H, W, KC, P)
    reg[:, :, :, :, 1:57, 1:57] = xr.transpose(0, 4, 5, 1, 2, 3)
    return xt, wt


def finish_output(results) -> np.ndarray:
    o_all = np.stack([results[c]["o"] for c in range(N_CORES)])
    # [core, mc, p, img, y, x] -> [core, img, y, x, mc, p]
    out = np.ascontiguousarray(
        o_all.transpose(0, 3, 4, 5, 1, 2).reshape(32, H, W, 256)
    )
    return out


def kernel(x: np.ndarray, kernel: np.ndarray) -> np.ndarray:
    from concourse.bass_utils import run_bass_kernel_spmd

    xt, wt = prep_inputs(x, kernel)
    nc = _get_nc()
    in_maps = [{"xt": xt[c], "wt": wt} for c in range(N_CORES)]
    res = run_bass_kernel_spmd(nc, in_maps, list(range(N_CORES)))
    return finish_output(res.results)


# revision 14
# speedup vs baseline: 1.0268x; 1.0268x over previous
"""BinaryConv2D (3x3 SAME, stride 1, NHWC/HWIO) on 8 Trainium2 NeuronCores.

Strategy
--------
Data-parallel over batch: 32 images -> 4 per core. Weights are binarized
host-side to +/-1 and replicated.

Per core the conv is computed as 18 accumulating float32r matmuls per
output tile (9 taps x 2 input-channel chunks of 128): the binarized
weight chunk [cin=128, cout=128] is the stationary operand, and a
2D-strided window of the channels-first padded image (8 output rows x 56
valid columns, row stride 58) is the moving operand. PSUM accumulates in
fp32; float32r streams at 1 row/cycle for moving dim >= 256 (4x faster
than fp32, ~1.1e-4 rel err for this 2304-term contraction on HW).

Host-side layout work (sharding prep): pad each image to 58x58 with a
zero ring, transpose to channels-first, flatten (img, y, x). A tap shift
is then a single scalar offset into the flat pixel axis, and every tap
read stays inside the padded image. The device output is channels-first
[cout_chunk, 128, img, y, x]; the host transposes back to NHWC.

Head-latency hiding: PE warmup matmuls on scratch data run while inputs
stream in (HAM un-throttles before real work), weights load on the ACT
HW-DGE ring concurrently with x on the SP ring, and image 0 arrives in
two halves so compute starts after ~1MB.
"""
import sys

sys.path.insert(0, "/opt/trn_rl_repo")

import numpy as np

P = 128
H = W = 56
HP = WP = 58                  # padded spatial dims
PIX = HP * WP                 # 3364 padded pixels per image
N_CORES = 8
IMGS = 4                      # images per core
FREE = IMGS * PIX             # 13456
ROWS = 8                      # output rows per tile
TILES = H // ROWS             # 7 tiles per image per cout chunk
NT = ROWS * W                 # 448 moving elements per matmul
KC = 2                        # cin chunks of 128
MC = 2                        # cout chunks of 128
TAPS = 9
WARMUP_MM = 12

_nc_cache = None


def _build_nc():
    import concourse.bacc as bacc
    import concourse.mybir as mybir
    from concourse import tile

    nc = bacc.Bacc("TRN2", target_bir_lowering=False, debug=False)
    f32r = mybir.dt.float32r
    f32 = mybir.dt.float32

    xt_d = nc.dram_tensor("xt", [KC, P, FREE], f32r, kind="ExternalInput")
    wt_d = nc.dram_tensor("wt", [P, MC, KC, TAPS, P], f32r, kind="ExternalInput")
    o_d = nc.dram_tensor("o", [MC, P, IMGS, H, W], f32, kind="ExternalOutput")

    with tile.TileContext(nc) as tc:
        with (
            tc.tile_pool(name="sb", bufs=1) as sb,
            tc.tile_pool(name="ps", bufs=1, space="PSUM") as ps,
        ):
            # PE warmup on zeroed scratch while input DMAs run: HAM
            # un-throttles to full clock before the real matmuls start.
            scratch = sb.tile([P, 512], f32r)
            nc.gpsimd.memset(scratch[:].bitcast(f32), 0.0)
            # One long accumulation group: back-to-back PE activity (no
            # inter-matmul sem roundtrips) so HAM un-throttles during the
            # input DMA head.
            warm = ps.tile([P, 512], f32, name="pt", bufs=8)
            for k in range(WARMUP_MM):
                nc.tensor.matmul(warm[:], scratch[:, :P], scratch[:],
                                 start=(k == 0), stop=(k == WARMUP_MM - 1),
                                 skip_group_check=True)

            w_sb = sb.tile([P, MC, KC, TAPS, P], f32r)
            x_sb = sb.tile([P, KC, FREE], f32r)
            # x on the SP ring in arrival-order pieces: rows 0-25 of
            # image 0 (tiles t=0..2) first, then the rest, then whole
            # images. Weights stream concurrently on the ACT ring in
            # consumption order; with the (img, mc, t) group order below,
            # mc1 weights have ~25us of slack.
            spans = [(0, 10 * WP), (10 * WP, 26 * WP), (26 * WP, PIX)] + [
                (i * PIX, (i + 1) * PIX) for i in range(1, IMGS)
            ]
            for lo, hi in spans:
                nc.sync.dma_start(
                    x_sb[:, :, lo:hi],
                    xt_d[:, :, lo:hi].rearrange("kc p q -> p kc q"),
                )
            for mc in range(MC):
                for kc in range(KC):
                    nc.scalar.dma_start(w_sb[:, mc, kc], wt_d[:, mc, kc])

            for i in range(IMGS):
                for mc in range(MC):
                    for t in range(TILES):
                        pt = ps.tile([P, ROWS, W], f32, name="pt", bufs=8)
                        k = 0
                        for kc in range(KC):
                            for tap in range(TAPS):
                                dy, dx = tap // 3, tap % 3
                                q0 = i * PIX + (ROWS * t + dy) * WP
                                rhs = (
                                    x_sb[:, kc, q0:q0 + ROWS * WP]
                                    .rearrange("p (r c) -> p r c", r=ROWS, c=WP)
                                    [:, :, dx:dx + W]
                                )
                                nc.tensor.matmul(
                                    pt[:],
                                    w_sb[:, mc, kc, tap, :],
                                    rhs,
                                    start=(k == 0),
                                    stop=(k == TAPS * KC - 1),
                                )
                                k += 1
                        o_t = sb.tile([P, ROWS, W], f32, name="ot", bufs=4)
                        nc.vector.tensor_copy(o_t[:], pt[:])
                        nc.sync.dma_start(
                            o_d[mc, :, i, ROWS * t:ROWS * (t + 1), :],
                            o_t[:],
                        )
    nc.compile()
    return nc


def _get_nc():
    global _nc_cache
    if _nc_cache is None:
        _nc_cache = _build_nc()
    return _nc_cache


def prep_inputs(x: np.ndarray, kernel: np.ndarray):
    """Host-side sharding/layout prep shared by kernel() and test timing."""
    x = np.ascontiguousarray(x, dtype=np.float32)
    kernel = np.ascontiguousarray(kernel, dtype=np.float32)
    assert x.shape == (32, H, W, 256) and kernel.shape == (3, 3, 256, 256)

    wb = np.where(kernel >= 0, np.float32(1.0), np.float32(-1.0))
    wt = (
        wb.reshape(TAPS, KC, P, MC, P)      # [tap, kc, p, mc, m]
        .transpose(2, 3, 1, 0, 4)           # [p, mc, kc, tap, m]
        .copy()
    )

    xt = np.zeros((N_CORES, KC, P, FREE), np.float32)
    reg = xt.reshape(N_CORES, KC, P, IMGS, HP, WP)
    xr = x.reshape(N_CORES, IMGS, H, W, KC, P)
    reg[:, :, :, :, 1:57, 1:57] = xr.transpose(0, 4, 5, 1, 2, 3)
    return xt, wt


def finish_output(results) -> np.ndarray:
    o_all = np.stack([results[c]["o"] for c in range(N_CORES)])
    # [core, mc, p, img, y, x] -> [core, img, y, x, mc, p]
    out = np.ascontiguousarray(
        o_all.transpose(0, 3, 4, 5, 1, 2).reshape(32, H, W, 256)
    )
    return out


def kernel(x: np.ndarray, kernel: np.ndarray) -> np.ndarray:
    from concourse.bass_utils import run_bass_kernel_spmd

    xt, wt = prep_inputs(x, kernel)
    nc = _get_nc()
    in_maps = [{"xt": xt[c], "wt": wt} for c in range(N_CORES)]
    res = run_bass_kernel_spmd(nc, in_maps, list(range(N_CORES)))
    return finish_output(res.results)
